# revision 27
# baseline (speedup 1.0000x reference)
"""Trainium2 kernel for nn_ColorLoss (retrieval_knn).

Computes mean_{b,m} min_n ||pred[b,m] - gt[b,n]|| for B=4, M=N=8192, D=3.

Strategy (8 NeuronCores, SPMD):
  - Shard queries over (batch, half-of-M): core c handles b = c//2,
    queries [h*4096, (h+1)*4096) with h = c%2, against the full gt[b].
  - K=26 augmented SPLIT-PRECISION bf16 matmul produces d2 DIRECTLY in
    PSUM.  fp32 matmuls cost 4 cycles/column on TRN2 ("2 half-speed
    matmuls") which made the PE the bottleneck (~440us serial); bf16 runs
    at 1 cycle/column with K-independent cost, so each coordinate is
    split hi+mid into two bf16 parts (q ~ qh+qm to ~17 mantissa bits)
    and the exact cross products are carried as separate K rows:
        per coord c: (qc_h^2 + qc_m-part | ones), (ones | gc parts),
                     (-2qc_h,-2qc_m) x (gc_h, gc_m)  [4 cross rows]
    plus 2 correction rows for the a2/b2 split residues.  Every bf16 x
    bf16 product is exact in fp32; rows are ordered so the running PSUM
    partial telescopes to ~(qc-gc)^2 per coordinate and stays small,
    keeping accumulation rounding at the fp32-reference level.
    (4-way PE row-group tiling at partition bases {0,32,64,96} keeps the
    matmuls concurrent; PE is now far off the critical path.)
  - The min-reduce is the bottleneck: every element must cross a PSUM
    read port at 1 elem/lane/cycle.  To halve the DVE's element count,
    the ScalarE (ACT) copies every other d2 slice PSUM->SBUF (1x rate),
    and the DVE consumes slices in PAIRS with a custom fused DVE op
        body = min(Src0_psum, Src1_sbuf); accum = min(body), init=BIG
    which reads one PSUM element + one SBUF element per cycle -> the DVE
    does E/2 cycles instead of E.  DVE and ACT run concurrently.
    (The stock TENSOR_TENSOR_REDUCE opcode faults this runtime - probed -
    so the op is registered through the custom-DVE table machinery.)
  - Per-pair accum mins [128, 1] land in mins_all; a final tensor_reduce
    collapses them to per-(m-tile) d2 mins [128, 32], DMA'd out.
  - Host: dist = sqrt(max(d2min, 0)) in float64, mean over all cores.
"""

import numpy as np

B, M, N, D = 4, 8192, 8192, 3
N_CORES = 8
MPC = (B * M) // N_CORES  # 4096 queries per core
M_TILES = MPC // 128  # 32
FD = 1024  # slice free size (2 psum banks)
PAIRS = N // (2 * FD)  # 4 pair-ops per m-tile (each covers 2*FD of n)
N_CHUNK = 512  # one matmul / one psum bank
K_ROWS = 26  # split-precision feature rows
LOSS_WEIGHT = 1.0
BIG = 3.0e38

_CACHE: dict = {}


def _register_pairmin_op():
    """body = min(Src0, Src1); accum_out = min(imm2, min over body outputs).

    Runtime-registered through the custom-DVE table (same machinery the
    previous revision of this kernel used for its d2+min op)."""
    import concourse.dve_ops as dops
    from concourse.dve_spec import C2, Spec, Src0, Src1, lower, minn
    from concourse.dve_uop import DveOpSpec

    name = "COLORLOSS_PAIRMIN_ANT"
    for o in dops.OPS:
        if o.name == name:
            return o

    def _ref(in0, in1, s0, s1, imm2):
        b = np.minimum(in0, in1).astype(np.float32)
        acc = np.minimum(
            np.float32(imm2), b.reshape(b.shape[0], -1).min(axis=-1, keepdims=True)
        ).astype(np.float32)
        return b, acc

    spec = Spec(body=minn(Src0, Src1), accum=minn, accum_init=C2, reference=_ref)
    row = dops._CUSTOM_DVE_ROW_BASE + len(dops.OPS)
    assert row < 0x20, "custom DVE row overflow"
    shas = {}
    for ver in ("v3", "v4"):
        s = DveOpSpec(name=name, opcode=row, uops=lower(spec, ver=ver), rd1_en=True)
        shas[ver] = s.sha(ver)
    op = dops.DveOp(name, spec, subdim=False, uops_sha=shas)
    dops.OPS.append(op)
    dops._SUB_OPCODE_FOR_NAME[name] = row
    dops.CUSTOM_DVE_SPECS[name] = spec  # keep the CoreSim lookup in sync
    return op


def _build_module(reps: int | None = None, ablation: str = "full"):
    """Build the SPMD module. reps=None is the production build; reps=R wraps
    the compute body in a For_i loop running it R times (timing builds).
    ablation: "full" | "dve_only" (no ACT copies; custom op reads a constant
    SBUF tile) | "act_only" (copies but no DVE pair ops) — timing probes;
    results are garbage for != "full"."""
    from contextlib import ExitStack

    import concourse.mybir as mybir
    import concourse.tile as tile
    from concourse import bacc

    pairmin_op = _register_pairmin_op()
    nc = bacc.Bacc(
        "TRN2", target_bir_lowering=False, debug=False, num_devices=N_CORES
    )
    f32 = mybir.dt.float32
    bf16 = mybir.dt.bfloat16
    qf_d = nc.dram_tensor("qf", [K_ROWS, MPC], bf16, kind="ExternalInput").ap()
    gf_d = nc.dram_tensor("gf", [K_ROWS, N], bf16, kind="ExternalInput").ap()
    dmin_d = nc.dram_tensor("dmin", [128, M_TILES], f32, kind="ExternalOutput").ap()

    with tile.TileContext(nc) as tc:
        with ExitStack() as ctx:
            inp = ctx.enter_context(tc.tile_pool(name="inp", bufs=1))
            pair_ps = ctx.enter_context(tc.tile_pool(name="pp", bufs=2, space="PSUM"))
            copy_ps = ctx.enter_context(tc.tile_pool(name="cp", bufs=2, space="PSUM"))
            # Generous depth: the DVE op's out-write extends each sbuf buf's
            # lifetime, so shallow pools would stall the ACT copy pipeline.
            sbufc = ctx.enter_context(tc.tile_pool(name="sc", bufs=8))
            small = ctx.enter_context(tc.tile_pool(name="sm", bufs=2))

            # qf/gf replicated at partition bases {0,32,64,96}: each chunk's
            # K=26 matmul runs in its own 32-row group (4 concurrent tiles).
            qf_sb = inp.tile([128, MPC], bf16)
            gf_sb = inp.tile([128, N], bf16)
            for i in range(4):
                nc.sync.dma_start(qf_sb[32 * i : 32 * i + K_ROWS, :], qf_d[:])
                nc.sync.dma_start(gf_sb[32 * i : 32 * i + K_ROWS, :], gf_d[:])
            def body():
                _emit_body(nc, tc, mybir, pairmin_op, qf_sb, gf_sb, dmin_d,
                           pair_ps, copy_ps, sbufc, small, ablation)

            if reps is None:
                body()
            else:
                with tc.For_i(0, reps, 1):
                    body()

    nc.compile()
    return nc


def _emit_body(nc, tc, mybir, pairmin_op, qf_sb, gf_sb, dmin_d, pair_ps, copy_ps,
               sbufc, small, ablation="full"):
    f32 = mybir.dt.float32
    mins_all = small.tile([128, M_TILES * PAIRS], f32, tag="mins_all")
    chunk_i = 0  # global matmul chunk counter -> PE row-group cycling

    def emit_d2(pt, mi, n0):
        """K=26 matmul of d2 for queries [mi*128,+128) x gt [n0, n0+FD)."""
        nonlocal chunk_i
        for c in range(FD // N_CHUNK):
            g = chunk_i % 4
            chunk_i += 1
            nn = n0 + c * N_CHUNK
            nc.tensor.matmul(
                pt[:, c * N_CHUNK : (c + 1) * N_CHUNK],
                qf_sb[32 * g : 32 * g + K_ROWS, mi * 128 : (mi + 1) * 128],
                gf_sb[32 * g : 32 * g + K_ROWS, nn : nn + N_CHUNK],
                start=True,
                stop=True,
                tile_position=(32 * g, 0),
            )

    for mi in range(M_TILES):
        for j in range(PAIRS):
            # copy-role slice first so the ACT copy overlaps the PE's
            # pair-role fill and the DVE's previous pair-op.
            cp = copy_ps.tile([128, FD], f32, tag="cp")
            emit_d2(cp[:], mi, (2 * j + 1) * FD)
            sb = sbufc.tile([128, FD], f32, tag="sb")
            if ablation != "dve_only":  # dve_only: in1 reads garbage, timing-true
                nc.scalar.copy(sb[:], cp[:])
            pp = pair_ps.tile([128, FD], f32, tag="pp")
            emit_d2(pp[:], mi, (2 * j) * FD)
            if ablation != "act_only":
                nc.vector._custom_dve(
                    pairmin_op,
                    # out lands in-place over the (consumed) sbuf copy: the
                    # psum pair buf is released at read-complete, so the PE
                    # can refill it without waiting on a PSUM write-drain.
                    out=sb[:],
                    in0=pp[:],
                    in1=sb[:],
                    imm2=BIG,
                    accum_out=mins_all[:, mi * PAIRS + j : mi * PAIRS + j + 1],
                )
    if ablation == "act_only":
        nc.gpsimd.memset(mins_all[:], 0.0)
    dmin = small.tile([128, M_TILES], f32, tag="dmin")
    nc.vector.tensor_reduce(
        dmin[:],
        mins_all[:].rearrange("p (m j) -> p m j", j=PAIRS),
        axis=mybir.AxisListType.X,
        op=mybir.AluOpType.min,
    )
    nc.sync.dma_start(dmin_d[:], dmin[:])


def _split2(x64):
    """float64 array -> (hi, mid) bf16 parts + float64 residual."""
    import ml_dtypes

    bf = ml_dtypes.bfloat16
    h = x64.astype(bf)
    r = x64 - h.astype(np.float64)
    m = r.astype(bf)
    r2 = r - m.astype(np.float64)
    return h, m, r2


def _features(pts64, side):
    """Split-precision feature rows for one side.

    side="q": rows carry (a2 parts | -2*coord parts | corr | ones)
    side="g": rows carry (ones | coord parts | b2 parts | corr)
    Row k of q-features pairs with row k of g-features so that
    sum_k qf[k]*gf[k] = |q~ - g~|^2 with q~,g~ the 17-bit-rounded points.
    Per-coordinate blocks keep the PSUM running partial telescoping small.
    """
    import ml_dtypes

    bf = ml_dtypes.bfloat16
    npts = pts64.shape[0]
    ch, cm, _ = _split2(pts64)  # [npts, 3] bf16 coordinate parts
    ct = ch.astype(np.float64) + cm.astype(np.float64)  # rounded points
    sq = ct * ct  # per-coordinate squares, float64
    sqh, sqm, sqr = _split2(sq)
    corr = sqr.sum(axis=1).astype(bf)  # split residue of the squares
    ones = np.ones(npts, dtype=bf)
    rows = []
    for c in range(3):
        if side == "q":
            blk = [
                sqh[:, c], sqm[:, c], ones, ones,
                (-2.0 * ch[:, c].astype(np.float64)).astype(bf),
                (-2.0 * ch[:, c].astype(np.float64)).astype(bf),
                (-2.0 * cm[:, c].astype(np.float64)).astype(bf),
                (-2.0 * cm[:, c].astype(np.float64)).astype(bf),
            ]
        else:
            blk = [
                ones, ones, sqh[:, c], sqm[:, c],
                ch[:, c], cm[:, c], ch[:, c], cm[:, c],
            ]
        rows.extend(blk)
    if side == "q":
        rows.extend([corr, ones])
    else:
        rows.extend([ones, corr])
    return np.stack(rows, axis=0)  # [K_ROWS, npts] bf16


def _prep_in_maps(pred_colors: np.ndarray, gt_colors: np.ndarray):
    pred_colors = np.asarray(pred_colors, dtype=np.float64)
    gt_colors = np.asarray(gt_colors, dtype=np.float64)
    in_maps = []
    gfs = [_features(gt_colors[b], "g") for b in range(B)]
    for c in range(N_CORES):
        b, h = divmod(c, N_CORES // B)
        q = pred_colors[b, h * MPC : (h + 1) * MPC]  # [MPC, 3]
        in_maps.append({"qf": _features(q, "q"), "gf": gfs[b]})
    return in_maps


def _get_module(reps: int | None = None):
    key = ("nc", reps)
    if key not in _CACHE:
        _CACHE[key] = _build_module(reps)
    return _CACHE[key]


def _finish(dmins: np.ndarray) -> np.ndarray:
    """dmins: [N_CORES, 128, M_TILES] per-query min squared distances."""
    d2 = np.asarray(dmins, dtype=np.float64)
    dist = np.sqrt(np.maximum(d2, 0.0))
    return np.asarray(dist.mean() * LOSS_WEIGHT, dtype=np.float32)


def kernel(pred_colors: np.ndarray, gt_colors: np.ndarray) -> np.ndarray:
    import time

    from concourse.bass_utils import run_bass_kernel_spmd

    nc = _get_module()
    in_maps = _prep_in_maps(pred_colors, gt_colors)
    last_err = None
    for attempt in range(3):  # first call after an unclean prior process can
        try:                  # hit a transient "device unrecoverable"; retry
            res = run_bass_kernel_spmd(nc, in_maps, core_ids=list(range(N_CORES)))
            break
        except Exception as e:  # noqa: BLE001
            last_err = e
            time.sleep(2.0)
            try:  # a fresh PJRT client clears terminal-side device state
                import jax

                jax.clear_backends()
            except Exception:  # noqa: BLE001
                pass
    else:
        raise last_err
    dmins = np.stack([res.results[c]["dmin"] for c in range(N_CORES)])
    return _finish(dmins)


# revision 28
# speedup vs baseline: 1.2245x; 1.2245x over previous
"""Trainium2 kernel for nn_ColorLoss (retrieval_knn).

Computes mean_{b,m} min_n ||pred[b,m] - gt[b,n]|| for B=4, M=N=8192, D=3.

Strategy (8 NeuronCores, SPMD):
  - Shard queries over (batch, half-of-M): core c handles b = c//2,
    queries [h*4096, (h+1)*4096) with h = c%2, against the full gt[b].
  - K=26 augmented SPLIT-PRECISION bf16 matmul produces d2 DIRECTLY in
    PSUM.  fp32 matmuls cost 4 cycles/column on TRN2 ("2 half-speed
    matmuls") which made the PE the bottleneck (~440us serial); bf16 runs
    at 1 cycle/column with K-independent cost, so each coordinate is
    split hi+mid into two bf16 parts (q ~ qh+qm to ~17 mantissa bits)
    and the exact cross products are carried as separate K rows:
        per coord c: (qc_h^2 + qc_m-part | ones), (ones | gc parts),
                     (-2qc_h,-2qc_m) x (gc_h, gc_m)  [4 cross rows]
    plus 2 correction rows for the a2/b2 split residues.  Every bf16 x
    bf16 product is exact in fp32; rows are ordered so the running PSUM
    partial telescopes to ~(qc-gc)^2 per coordinate and stays small,
    keeping accumulation rounding at the fp32-reference level.
    (4-way PE row-group tiling at partition bases {0,32,64,96} keeps the
    matmuls concurrent; PE is now far off the critical path.)
  - The min-reduce is the bottleneck: every element must cross a PSUM
    read port at 1 elem/lane/cycle.  To halve the DVE's element count,
    the ScalarE (ACT) copies every other d2 slice PSUM->SBUF (1x rate),
    and the DVE consumes slices in PAIRS with a custom fused DVE op
        body = min(Src0_psum, Src1_sbuf); accum = min(body), init=BIG
    which reads one PSUM element + one SBUF element per cycle -> the DVE
    does E/2 cycles instead of E.  DVE and ACT run concurrently.
    (The stock TENSOR_TENSOR_REDUCE opcode faults this runtime - probed -
    so the op is registered through the custom-DVE table machinery.)
  - Per-pair accum mins [128, 1] land in mins_all; a final tensor_reduce
    collapses them to per-(m-tile) d2 mins [128, 32], DMA'd out.
  - Host: dist = sqrt(max(d2min, 0)) in float64, mean over all cores.
"""

import numpy as np

B, M, N, D = 4, 8192, 8192, 3
N_CORES = 8
MPC = (B * M) // N_CORES  # 4096 queries per core
M_TILES = MPC // 128  # 32
FD = 1024  # slice free size (2 psum banks)
PAIRS = N // (2 * FD)  # 4 pair-ops per m-tile (each covers 2*FD of n)
N_CHUNK = 512  # one matmul / one psum bank
K_ROWS = 26  # split-precision feature rows
LOSS_WEIGHT = 1.0
BIG = 3.0e38

_CACHE: dict = {}


def _register_pairmin_op():
    """body = min(Src0, Src1); accum_out = min(imm2, min over body outputs).

    Runtime-registered through the custom-DVE table (same machinery the
    previous revision of this kernel used for its d2+min op)."""
    import concourse.dve_ops as dops
    from concourse.dve_spec import C2, Spec, Src0, Src1, lower, minn
    from concourse.dve_uop import DveOpSpec

    name = "COLORLOSS_PAIRMIN_ANT"
    for o in dops.OPS:
        if o.name == name:
            return o

    def _ref(in0, in1, s0, s1, imm2):
        b = np.minimum(in0, in1).astype(np.float32)
        acc = np.minimum(
            np.float32(imm2), b.reshape(b.shape[0], -1).min(axis=-1, keepdims=True)
        ).astype(np.float32)
        return b, acc

    spec = Spec(body=minn(Src0, Src1), accum=minn, accum_init=C2, reference=_ref)
    row = dops._CUSTOM_DVE_ROW_BASE + len(dops.OPS)
    assert row < 0x20, "custom DVE row overflow"
    shas = {}
    for ver in ("v3", "v4"):
        s = DveOpSpec(name=name, opcode=row, uops=lower(spec, ver=ver), rd1_en=True)
        shas[ver] = s.sha(ver)
    op = dops.DveOp(name, spec, subdim=False, uops_sha=shas)
    dops.OPS.append(op)
    dops._SUB_OPCODE_FOR_NAME[name] = row
    dops.CUSTOM_DVE_SPECS[name] = spec  # keep the CoreSim lookup in sync
    return op


def _build_module(reps: int | None = None, ablation: str = "full"):
    """Build the SPMD module. reps=None is the production build; reps=R wraps
    the compute body in a For_i loop running it R times (timing builds).
    ablation: "full" | "dve_only" (no ACT copies; custom op reads a constant
    SBUF tile) | "act_only" (copies but no DVE pair ops) — timing probes;
    results are garbage for != "full"."""
    from contextlib import ExitStack

    import concourse.mybir as mybir
    import concourse.tile as tile
    from concourse import bacc

    pairmin_op = _register_pairmin_op()
    nc = bacc.Bacc(
        "TRN2", target_bir_lowering=False, debug=False, num_devices=N_CORES
    )
    f32 = mybir.dt.float32
    bf16 = mybir.dt.bfloat16
    qf_d = nc.dram_tensor("qf", [K_ROWS, MPC], bf16, kind="ExternalInput").ap()
    gf_d = nc.dram_tensor("gf", [K_ROWS, N], bf16, kind="ExternalInput").ap()
    dmin_d = nc.dram_tensor("dmin", [128, M_TILES], f32, kind="ExternalOutput").ap()

    with tile.TileContext(nc) as tc:
        with ExitStack() as ctx:
            inp = ctx.enter_context(tc.tile_pool(name="inp", bufs=1))
            pair_ps = ctx.enter_context(tc.tile_pool(name="pp", bufs=2, space="PSUM"))
            copy_ps = ctx.enter_context(tc.tile_pool(name="cp", bufs=2, space="PSUM"))
            sbufc = ctx.enter_context(tc.tile_pool(name="sc", bufs=4))
            small = ctx.enter_context(tc.tile_pool(name="sm", bufs=2))

            # qf/gf replicated at partition bases {0,32,64,96}: each chunk's
            # K=26 matmul runs in its own 32-row group (4 concurrent tiles).
            qf_sb = inp.tile([128, MPC], bf16)
            gf_sb = inp.tile([128, N], bf16)
            for i in range(4):
                nc.sync.dma_start(qf_sb[32 * i : 32 * i + K_ROWS, :], qf_d[:])
                nc.sync.dma_start(gf_sb[32 * i : 32 * i + K_ROWS, :], gf_d[:])
            def body():
                _emit_body(nc, tc, mybir, pairmin_op, qf_sb, gf_sb, dmin_d,
                           pair_ps, copy_ps, sbufc, small, ablation)

            if reps is None:
                body()
            else:
                with tc.For_i(0, reps, 1):
                    body()

    nc.compile()
    return nc


def _emit_body(nc, tc, mybir, pairmin_op, qf_sb, gf_sb, dmin_d, pair_ps, copy_ps,
               sbufc, small, ablation="full"):
    f32 = mybir.dt.float32
    mins_all = small.tile([128, M_TILES * PAIRS], f32, tag="mins_all")
    chunk_i = 0  # global matmul chunk counter -> PE row-group cycling

    def emit_d2(pt, mi, n0):
        """K=26 matmul of d2 for queries [mi*128,+128) x gt [n0, n0+FD)."""
        nonlocal chunk_i
        for c in range(FD // N_CHUNK):
            g = chunk_i % 4
            chunk_i += 1
            nn = n0 + c * N_CHUNK
            nc.tensor.matmul(
                pt[:, c * N_CHUNK : (c + 1) * N_CHUNK],
                qf_sb[32 * g : 32 * g + K_ROWS, mi * 128 : (mi + 1) * 128],
                gf_sb[32 * g : 32 * g + K_ROWS, nn : nn + N_CHUNK],
                start=True,
                stop=True,
                tile_position=(32 * g, 0),
            )

    for mi in range(M_TILES):
        for j in range(PAIRS):
            # copy-role slice first so the ACT copy overlaps the PE's
            # pair-role fill and the DVE's previous pair-op.
            cp = copy_ps.tile([128, FD], f32, tag="cp")
            emit_d2(cp[:], mi, (2 * j + 1) * FD)
            sb = sbufc.tile([128, FD], f32, tag="sb")
            if ablation != "dve_only":  # dve_only: in1 reads garbage, timing-true
                nc.scalar.copy(sb[:], cp[:])
            pp = pair_ps.tile([128, FD], f32, tag="pp")
            emit_d2(pp[:], mi, (2 * j) * FD)
            if ablation != "act_only":
                nc.vector._custom_dve(
                    pairmin_op,
                    # out lands in-place over the (consumed) sbuf copy: the
                    # psum pair buf is released at read-complete, so the PE
                    # can refill it without waiting on a PSUM write-drain.
                    out=sb[:],
                    in0=pp[:],
                    in1=sb[:],
                    imm2=BIG,
                    accum_out=mins_all[:, mi * PAIRS + j : mi * PAIRS + j + 1],
                )
    if ablation == "act_only":
        nc.gpsimd.memset(mins_all[:], 0.0)
    dmin = small.tile([128, M_TILES], f32, tag="dmin")
    nc.vector.tensor_reduce(
        dmin[:],
        mins_all[:].rearrange("p (m j) -> p m j", j=PAIRS),
        axis=mybir.AxisListType.X,
        op=mybir.AluOpType.min,
    )
    nc.sync.dma_start(dmin_d[:], dmin[:])


def _split2(x64):
    """float64 array -> (hi, mid) bf16 parts + float64 residual."""
    import ml_dtypes

    bf = ml_dtypes.bfloat16
    h = x64.astype(bf)
    r = x64 - h.astype(np.float64)
    m = r.astype(bf)
    r2 = r - m.astype(np.float64)
    return h, m, r2


def _features(pts64, side):
    """Split-precision feature rows for one side.

    side="q": rows carry (a2 parts | -2*coord parts | corr | ones)
    side="g": rows carry (ones | coord parts | b2 parts | corr)
    Row k of q-features pairs with row k of g-features so that
    sum_k qf[k]*gf[k] = |q~ - g~|^2 with q~,g~ the 17-bit-rounded points.
    Per-coordinate blocks keep the PSUM running partial telescoping small.
    """
    import ml_dtypes

    bf = ml_dtypes.bfloat16
    npts = pts64.shape[0]
    ch, cm, _ = _split2(pts64)  # [npts, 3] bf16 coordinate parts
    ct = ch.astype(np.float64) + cm.astype(np.float64)  # rounded points
    sq = ct * ct  # per-coordinate squares, float64
    sqh, sqm, sqr = _split2(sq)
    corr = sqr.sum(axis=1).astype(bf)  # split residue of the squares
    ones = np.ones(npts, dtype=bf)
    rows = []
    for c in range(3):
        if side == "q":
            blk = [
                sqh[:, c], sqm[:, c], ones, ones,
                (-2.0 * ch[:, c].astype(np.float64)).astype(bf),
                (-2.0 * ch[:, c].astype(np.float64)).astype(bf),
                (-2.0 * cm[:, c].astype(np.float64)).astype(bf),
                (-2.0 * cm[:, c].astype(np.float64)).astype(bf),
            ]
        else:
            blk = [
                ones, ones, sqh[:, c], sqm[:, c],
                ch[:, c], cm[:, c], ch[:, c], cm[:, c],
            ]
        rows.extend(blk)
    if side == "q":
        rows.extend([corr, ones])
    else:
        rows.extend([ones, corr])
    return np.stack(rows, axis=0)  # [K_ROWS, npts] bf16


def _prep_in_maps(pred_colors: np.ndarray, gt_colors: np.ndarray):
    pred_colors = np.asarray(pred_colors, dtype=np.float64)
    gt_colors = np.asarray(gt_colors, dtype=np.float64)
    in_maps = []
    gfs = [_features(gt_colors[b], "g") for b in range(B)]
    for c in range(N_CORES):
        b, h = divmod(c, N_CORES // B)
        q = pred_colors[b, h * MPC : (h + 1) * MPC]  # [MPC, 3]
        in_maps.append({"qf": _features(q, "q"), "gf": gfs[b]})
    return in_maps


def _get_module(reps: int | None = None):
    key = ("nc", reps)
    if key not in _CACHE:
        _CACHE[key] = _build_module(reps)
    return _CACHE[key]


def _finish(dmins: np.ndarray) -> np.ndarray:
    """dmins: [N_CORES, 128, M_TILES] per-query min squared distances."""
    d2 = np.asarray(dmins, dtype=np.float64)
    dist = np.sqrt(np.maximum(d2, 0.0))
    return np.asarray(dist.mean() * LOSS_WEIGHT, dtype=np.float32)


def kernel(pred_colors: np.ndarray, gt_colors: np.ndarray) -> np.ndarray:
    import time

    from concourse.bass_utils import run_bass_kernel_spmd

    nc = _get_module()
    in_maps = _prep_in_maps(pred_colors, gt_colors)
    last_err = None
    for attempt in range(3):  # first call after an unclean prior process can
        try:                  # hit a transient "device unrecoverable"; retry
            res = run_bass_kernel_spmd(nc, in_maps, core_ids=list(range(N_CORES)))
            break
        except Exception as e:  # noqa: BLE001
            last_err = e
            time.sleep(2.0)
            try:  # a fresh PJRT client clears terminal-side device state
                import jax

                jax.clear_backends()
            except Exception:  # noqa: BLE001
                pass
    else:
        raise last_err
    dmins = np.stack([res.results[c]["dmin"] for c in range(N_CORES)])
    return _finish(dmins)


# revision 29
# speedup vs baseline: 1.2371x; 1.0103x over previous
"""Trainium2 kernel for nn_ColorLoss (retrieval_knn).

Computes mean_{b,m} min_n ||pred[b,m] - gt[b,n]|| for B=4, M=N=8192, D=3.

Strategy (8 NeuronCores, SPMD):
  - Shard queries over (batch, half-of-M): core c handles b = c//2,
    queries [h*4096, (h+1)*4096) with h = c%2, against the full gt[b].
  - K=26 augmented SPLIT-PRECISION bf16 matmul produces d2 DIRECTLY in
    PSUM.  fp32 matmuls cost 4 cycles/column on TRN2 ("2 half-speed
    matmuls") which made the PE the bottleneck (~440us serial); bf16 runs
    at 1 cycle/column with K-independent cost, so each coordinate is
    split hi+mid into two bf16 parts (q ~ qh+qm to ~17 mantissa bits)
    and the exact cross products are carried as separate K rows:
        per coord c: (qc_h^2 + qc_m-part | ones), (ones | gc parts),
                     (-2qc_h,-2qc_m) x (gc_h, gc_m)  [4 cross rows]
    plus 2 correction rows for the a2/b2 split residues.  Every bf16 x
    bf16 product is exact in fp32; rows are ordered so the running PSUM
    partial telescopes to ~(qc-gc)^2 per coordinate and stays small,
    keeping accumulation rounding at the fp32-reference level.
    (4-way PE row-group tiling at partition bases {0,32,64,96} keeps the
    matmuls concurrent; PE is now far off the critical path.)
  - The min-reduce is the bottleneck: every element must cross a PSUM
    read port at 1 elem/lane/cycle.  To halve the DVE's element count,
    the ScalarE (ACT) copies every other d2 slice PSUM->SBUF (1x rate),
    and the DVE consumes slices in PAIRS with a custom fused DVE op
        body = min(Src0_psum, Src1_sbuf); accum = min(body), init=BIG
    which reads one PSUM element + one SBUF element per cycle -> the DVE
    does E/2 cycles instead of E.  DVE and ACT run concurrently.
    (The stock TENSOR_TENSOR_REDUCE opcode faults this runtime - probed -
    so the op is registered through the custom-DVE table machinery.)
  - Per-pair accum mins [128, 1] land in mins_all; a final tensor_reduce
    collapses them to per-(m-tile) d2 mins [128, 32], DMA'd out.
  - Host: dist = sqrt(max(d2min, 0)) in float64, mean over all cores.
"""

import numpy as np

B, M, N, D = 4, 8192, 8192, 3
N_CORES = 8
MPC = (B * M) // N_CORES  # 4096 queries per core
M_TILES = MPC // 128  # 32
FD = 1024  # slice free size (2 psum banks)
PAIRS = N // (2 * FD)  # 4 pair-ops per m-tile (each covers 2*FD of n)
N_CHUNK = 512  # one matmul / one psum bank
K_ROWS = 26  # split-precision feature rows
LOSS_WEIGHT = 1.0
BIG = 3.0e38

_CACHE: dict = {}


def _register_pairmin_op():
    """body = min(Src0, Src1); accum_out = min(imm2, min over body outputs).

    Runtime-registered through the custom-DVE table (same machinery the
    previous revision of this kernel used for its d2+min op)."""
    import concourse.dve_ops as dops
    from concourse.dve_spec import C2, Spec, Src0, Src1, lower, minn
    from concourse.dve_uop import DveOpSpec

    name = "COLORLOSS_PAIRMIN_ANT"
    for o in dops.OPS:
        if o.name == name:
            return o

    def _ref(in0, in1, s0, s1, imm2):
        b = np.minimum(in0, in1).astype(np.float32)
        acc = np.minimum(
            np.float32(imm2), b.reshape(b.shape[0], -1).min(axis=-1, keepdims=True)
        ).astype(np.float32)
        return b, acc

    spec = Spec(body=minn(Src0, Src1), accum=minn, accum_init=C2, reference=_ref)
    row = dops._CUSTOM_DVE_ROW_BASE + len(dops.OPS)
    assert row < 0x20, "custom DVE row overflow"
    shas = {}
    for ver in ("v3", "v4"):
        s = DveOpSpec(name=name, opcode=row, uops=lower(spec, ver=ver), rd1_en=True)
        shas[ver] = s.sha(ver)
    op = dops.DveOp(name, spec, subdim=False, uops_sha=shas)
    dops.OPS.append(op)
    dops._SUB_OPCODE_FOR_NAME[name] = row
    dops.CUSTOM_DVE_SPECS[name] = spec  # keep the CoreSim lookup in sync
    return op


def _build_module(reps: int | None = None, ablation: str = "full"):
    """Build the SPMD module. reps=None is the production build; reps=R wraps
    the compute body in a For_i loop running it R times (timing builds).
    ablation: "full" | "dve_only" (no ACT copies; custom op reads a constant
    SBUF tile) | "act_only" (copies but no DVE pair ops) — timing probes;
    results are garbage for != "full"."""
    from contextlib import ExitStack

    import concourse.mybir as mybir
    import concourse.tile as tile
    from concourse import bacc

    pairmin_op = _register_pairmin_op()
    nc = bacc.Bacc(
        "TRN2", target_bir_lowering=False, debug=False, num_devices=N_CORES
    )
    f32 = mybir.dt.float32
    bf16 = mybir.dt.bfloat16
    qf_d = nc.dram_tensor("qf", [K_ROWS, MPC], bf16, kind="ExternalInput").ap()
    gf_d = nc.dram_tensor("gf", [K_ROWS, N], bf16, kind="ExternalInput").ap()
    dmin_d = nc.dram_tensor("dmin", [128, M_TILES], f32, kind="ExternalOutput").ap()

    with tile.TileContext(nc) as tc:
        with ExitStack() as ctx:
            inp = ctx.enter_context(tc.tile_pool(name="inp", bufs=1))
            pair_ps = ctx.enter_context(tc.tile_pool(name="pp", bufs=2, space="PSUM"))
            copy_ps = ctx.enter_context(tc.tile_pool(name="cp", bufs=2, space="PSUM"))
            sbufc = ctx.enter_context(tc.tile_pool(name="sc", bufs=4))
            small = ctx.enter_context(tc.tile_pool(name="sm", bufs=2))

            # qf/gf replicated at partition bases {0,32,64,96}: each chunk's
            # K=26 matmul runs in its own 32-row group (4 concurrent tiles).
            qf_sb = inp.tile([128, MPC], bf16)
            gf_sb = inp.tile([128, N], bf16)
            for i in range(4):
                nc.sync.dma_start(qf_sb[32 * i : 32 * i + K_ROWS, :], qf_d[:])
                nc.sync.dma_start(gf_sb[32 * i : 32 * i + K_ROWS, :], gf_d[:])
            def body():
                _emit_body(nc, tc, mybir, pairmin_op, qf_sb, gf_sb, dmin_d,
                           pair_ps, copy_ps, sbufc, small, ablation)

            if reps is None:
                body()
            else:
                with tc.For_i(0, reps, 1):
                    body()

    nc.compile()
    return nc


def _emit_body(nc, tc, mybir, pairmin_op, qf_sb, gf_sb, dmin_d, pair_ps, copy_ps,
               sbufc, small, ablation="full"):
    f32 = mybir.dt.float32
    mins_all = small.tile([128, M_TILES * PAIRS], f32, tag="mins_all")
    chunk_i = 0  # global matmul chunk counter -> PE row-group cycling

    def emit_d2(pt, mi, n0):
        """K=26 matmul of d2 for queries [mi*128,+128) x gt [n0, n0+FD)."""
        nonlocal chunk_i
        for c in range(FD // N_CHUNK):
            g = chunk_i % 4
            chunk_i += 1
            nn = n0 + c * N_CHUNK
            nc.tensor.matmul(
                pt[:, c * N_CHUNK : (c + 1) * N_CHUNK],
                qf_sb[32 * g : 32 * g + K_ROWS, mi * 128 : (mi + 1) * 128],
                gf_sb[32 * g : 32 * g + K_ROWS, nn : nn + N_CHUNK],
                start=True,
                stop=True,
                tile_position=(32 * g, 0),
            )

    for mi in range(M_TILES):
        for j in range(PAIRS):
            # copy-role slice first so the ACT copy overlaps the PE's
            # pair-role fill and the DVE's previous pair-op.
            cp = copy_ps.tile([128, FD], f32, tag="cp")
            emit_d2(cp[:], mi, (2 * j + 1) * FD)
            sb = sbufc.tile([128, FD], f32, tag="sb")
            if ablation != "dve_only":  # dve_only: in1 reads garbage, timing-true
                nc.scalar.copy(sb[:], cp[:])
            pp = pair_ps.tile([128, FD], f32, tag="pp")
            emit_d2(pp[:], mi, (2 * j) * FD)
            if ablation != "act_only":
                nc.vector._custom_dve(
                    pairmin_op,
                    # out lands in-place over the (consumed) sbuf copy: the
                    # psum pair buf is released at read-complete, so the PE
                    # can refill it without waiting on a PSUM write-drain.
                    # ("outpp" variant writes back to psum instead - A/B probe.)
                    out=pp[:] if ablation == "outpp" else sb[:],
                    in0=pp[:],
                    in1=sb[:],
                    imm2=BIG,
                    accum_out=mins_all[:, mi * PAIRS + j : mi * PAIRS + j + 1],
                )
    if ablation == "act_only":
        nc.gpsimd.memset(mins_all[:], 0.0)
    dmin = small.tile([128, M_TILES], f32, tag="dmin")
    nc.vector.tensor_reduce(
        dmin[:],
        mins_all[:].rearrange("p (m j) -> p m j", j=PAIRS),
        axis=mybir.AxisListType.X,
        op=mybir.AluOpType.min,
    )
    nc.sync.dma_start(dmin_d[:], dmin[:])


def _split2(x64):
    """float64 array -> (hi, mid) bf16 parts + float64 residual."""
    import ml_dtypes

    bf = ml_dtypes.bfloat16
    h = x64.astype(bf)
    r = x64 - h.astype(np.float64)
    m = r.astype(bf)
    r2 = r - m.astype(np.float64)
    return h, m, r2


def _features(pts64, side):
    """Split-precision feature rows for one side.

    side="q": rows carry (a2 parts | -2*coord parts | corr | ones)
    side="g": rows carry (ones | coord parts | b2 parts | corr)
    Row k of q-features pairs with row k of g-features so that
    sum_k qf[k]*gf[k] = |q~ - g~|^2 with q~,g~ the 17-bit-rounded points.
    Per-coordinate blocks keep the PSUM running partial telescoping small.
    """
    import ml_dtypes

    bf = ml_dtypes.bfloat16
    npts = pts64.shape[0]
    ch, cm, _ = _split2(pts64)  # [npts, 3] bf16 coordinate parts
    ct = ch.astype(np.float64) + cm.astype(np.float64)  # rounded points
    sq = ct * ct  # per-coordinate squares, float64
    sqh, sqm, sqr = _split2(sq)
    corr = sqr.sum(axis=1).astype(bf)  # split residue of the squares
    ones = np.ones(npts, dtype=bf)
    rows = []
    for c in range(3):
        if side == "q":
            blk = [
                sqh[:, c], sqm[:, c], ones, ones,
                (-2.0 * ch[:, c].astype(np.float64)).astype(bf),
                (-2.0 * ch[:, c].astype(np.float64)).astype(bf),
                (-2.0 * cm[:, c].astype(np.float64)).astype(bf),
                (-2.0 * cm[:, c].astype(np.float64)).astype(bf),
            ]
        else:
            blk = [
                ones, ones, sqh[:, c], sqm[:, c],
                ch[:, c], cm[:, c], ch[:, c], cm[:, c],
            ]
        rows.extend(blk)
    if side == "q":
        rows.extend([corr, ones])
    else:
        rows.extend([ones, corr])
    return np.stack(rows, axis=0)  # [K_ROWS, npts] bf16


def _prep_in_maps(pred_colors: np.ndarray, gt_colors: np.ndarray):
    pred_colors = np.asarray(pred_colors, dtype=np.float64)
    gt_colors = np.asarray(gt_colors, dtype=np.float64)
    in_maps = []
    gfs = [_features(gt_colors[b], "g") for b in range(B)]
    for c in range(N_CORES):
        b, h = divmod(c, N_CORES // B)
        q = pred_colors[b, h * MPC : (h + 1) * MPC]  # [MPC, 3]
        in_maps.append({"qf": _features(q, "q"), "gf": gfs[b]})
    return in_maps


def _get_module(reps: int | None = None):
    key = ("nc", reps)
    if key not in _CACHE:
        _CACHE[key] = _build_module(reps)
    return _CACHE[key]


def _finish(dmins: np.ndarray) -> np.ndarray:
    """dmins: [N_CORES, 128, M_TILES] per-query min squared distances."""
    d2 = np.asarray(dmins, dtype=np.float64)
    dist = np.sqrt(np.maximum(d2, 0.0))
    return np.asarray(dist.mean() * LOSS_WEIGHT, dtype=np.float32)


def kernel(pred_colors: np.ndarray, gt_colors: np.ndarray) -> np.ndarray:
    import time

    from concourse.bass_utils import run_bass_kernel_spmd

    nc = _get_module()
    in_maps = _prep_in_maps(pred_colors, gt_colors)
    last_err = None
    for attempt in range(3):  # first call after an unclean prior process can
        try:                  # hit a transient "device unrecoverable"; retry
            res = run_bass_kernel_spmd(nc, in_maps, core_ids=list(range(N_CORES)))
            break
        except Exception as e:  # noqa: BLE001
            last_err = e
            time.sleep(2.0)
            try:  # a fresh PJRT client clears terminal-side device state
                import jax

                jax.clear_backends()
            except Exception:  # noqa: BLE001
                pass
    else:
        raise last_err
    dmins = np.stack([res.results[c]["dmin"] for c in range(N_CORES)])
    return _finish(dmins)
